# revision 6
# baseline (speedup 1.0000x reference)
"""Trainium2 Bass kernel for nn_A2EvULoss (EvU loss over [1M, 100] logits).

Data-parallel over 8 NeuronCores; each core streams its 125k-row shard once
from HBM (p-major layout: partition p holds rows p*976+c, giving 6.4KB
contiguous DMA bursts per partition).

Streaming phase (per 16-row-tile chunk), balanced so the DMA is the wall:
 - ScalarE: one batched exp over the whole chunk, output in bf16 (~1.5us).
 - GpSimd: pairwise add of the two class halves (bf16 in, f32 out) (~1.8us).
 - DVE: segmented reduce_sum of the pairwise sums gives per-row sumexp;
   a bf16 max chain (two 2x tensor_tensor max levels, the second using an
   overlapped split since max is idempotent, then a short reduce_max)
   gives the per-row max evidence exp(xmax) directly (~2.1us).
Max chains run SHIFT chunks behind the stream so the last few fill the
latency window of the umin/umax all-reduce after the stream ends.

Correctness test uses a host-side gather: xt[i] = x[i, target[i]] is fed
as a second (tiny) input; the row is correct iff bf16(exp(xt)) equals the
bf16 max of exp(x) (identical spline + rounding path, so equality is
exact for true argmax rows; bf16-tie false positives are ~7e-5 of rows).

Tail: all-reduce (max) of (max sumexp, -min sumexp) gives global umin/umax
(unc = C/(C+sumexp) is monotone); per-row weights go to four bf16 planes
(c-major [128, 992, 4]); 22 threshold masks are built per column chunk with
4x-mode tensor_scalar is_le against a bf16 bucket index in a group-major
layout; 62 PSUM-accumulated matmuls (G=16 stat columns each) produce a
block-diagonal [64, 352] PSUM; strided DMAs extract the [4, 16, 22]
diagonal, a reduce gives [4, 22], which is all-reduced; every core computes
the trapezoid AUC + -log replicated; core 0's scalar is returned.
"""

import numpy as np

P = 128
C = 100
H = C // 2                        # 50: pairwise split of the class dim
N_CORES = 8
N_TOTAL = 1_000_000
NPC = N_TOTAL // N_CORES          # 125000 rows per core
MCOLS = NPC // P                  # 976 main stat columns (p-major layout)
REM = NPC - P * MCOLS             # 72 remainder rows
COLS = MCOLS + 1                  # 977 stat columns (col 976 = remainder)
CP = 992                          # padded to a multiple of G=16 for matmuls
T = 16                            # row-tiles per streaming chunk
NCHUNKS = MCOLS // T              # 61
SHIFT = 5                         # max chains run SHIFT units behind stream
NTH = 21
K = NTH + 1                       # 21 thresholds + 1 all-ones (totals) column
G = 16                            # stat columns per matmul group
CW = 496                          # mask-build chunk width (31 groups; 2*496)
EPS = 1e-10


def _build_nc():
    import bass_rust
    import concourse.bass as bass
    import concourse.bacc as bacc
    import concourse.tile as tile
    from concourse import mybir

    f32 = mybir.dt.float32
    bf16 = mybir.dt.bfloat16
    Op = mybir.AluOpType
    Act = mybir.ActivationFunctionType
    X = mybir.AxisListType.X

    nc = bacc.Bacc("TRN2", target_bir_lowering=False, debug=False,
                   num_devices=N_CORES)

    x_d = nc.dram_tensor("x", [NPC, C], f32, kind="ExternalInput")
    xt_d = nc.dram_tensor("xt", [P, CP], f32, kind="ExternalInput")
    valid_d = nc.dram_tensor("valid", [P, CP], f32, kind="ExternalInput")
    out_d = nc.dram_tensor("out", [1, 1], f32, kind="ExternalOutput")

    x_main = x_d.ap()[0:P * MCOLS, :].rearrange("(p c) f -> p (c f)", p=P)
    x_rem = x_d.ap()[P * MCOLS:NPC, :]                      # [72, 100]

    with tile.TileContext(nc) as tc:
        with (
            tc.tile_pool(name="stream", bufs=3) as stream,
            tc.tile_pool(name="ystream", bufs=SHIFT + 2) as ystream,
            tc.tile_pool(name="persist", bufs=1) as persist,
            tc.tile_pool(name="psum", bufs=1, space="PSUM") as psump,
            tc.tile_pool(name="dram", bufs=1, space="DRAM") as dram,
        ):
            # ---- persistent inputs / stats ----
            xt_sb = persist.tile([P, CP], f32)
            nc.sync.dma_start(xt_sb[:], xt_d.ap())
            valid_sb = persist.tile([P, CP], f32)
            nc.sync.dma_start(valid_sb[:], valid_d.ap())

            pmax = persist.tile([P, CP], bf16)     # per-row max of exp(x)
            sumexp = persist.tile([P, CP], f32)
            nc.vector.memset(pmax[:, COLS:CP], 0.0)
            nc.vector.memset(sumexp[:, COLS:CP], 0.0)
            padb = persist.tile([P, 1], f32)
            se_rem = persist.tile([P, 2], f32)  # (min-in, max-in) of col 976

            c100 = persist.tile([P, 1], f32)
            nc.vector.memset(c100[:], float(C))

            # warm-up/sync collective: wakes the CC path and re-syncs core
            # skew mid-stream so the real all-reduces wait less
            warm_in = dram.tile([1, 2], f32)
            warm_out = dram.tile([1, 2], f32)

            # ---- phase 1: stream x, compute y=exp(x) bf16, max + sumexp ----
            def maxchain(y, sl, nt):
                yv = y[:, 0:nt * C].rearrange("p (t f) -> p t f", f=C)
                mx1 = stream.tile([P, T, H], bf16, tag="mx1")
                nc.vector.tensor_tensor(
                    mx1[:, 0:nt, :], yv[:, :, 0:H], yv[:, :, H:C], op=Op.max)
                # overlapped split (24:50 vs 0:26) keeps both operands
                # 4B-aligned; max is idempotent so the overlap is harmless
                mx2 = stream.tile([P, T, 26], bf16, tag="mx2")
                nc.vector.tensor_tensor(
                    mx2[:, 0:nt, :], mx1[:, 0:nt, 0:26], mx1[:, 0:nt, 24:H],
                    op=Op.max)
                nc.vector.reduce_max(pmax[:, sl], mx2[:, 0:nt, :], axis=X)

            # chunk 0 split into 4 small sub-chunks so compute starts as
            # soon as the first 200KB lands
            units = [(4 * i, 4, False) for i in range(4)]
            units += [(16 + T * i, T, False) for i in range(NCHUNKS - 1)]
            units += [(MCOLS, 1, True)]
            pending = {}
            for ui, (c0u, nt, last) in enumerate(units):
                xtile = stream.tile([P, T * C], f32, tag="xt")
                if last:
                    nc.vector.memset(xtile[:, 0:C], 0.0)
                    nc.sync.dma_start(xtile[0:REM, 0:C], x_rem)
                    sl = slice(MCOLS, COLS)
                else:
                    nc.sync.dma_start(
                        xtile[:, 0:nt * C],
                        x_main[:, c0u * C:(c0u + nt) * C])
                    sl = slice(c0u, c0u + nt)

                y = ystream.tile([P, T * C], bf16, tag="y")
                nc.scalar.activation(y[:, 0:nt * C], xtile[:, 0:nt * C],
                                     Act.Exp)

                yv = y[:, 0:nt * C].rearrange("p (t f) -> p t f", f=C)
                s1t = stream.tile([P, T, H], f32, tag="s1")
                nc.gpsimd.tensor_tensor(
                    s1t[:, 0:nt, :], yv[:, :, 0:H], yv[:, :, H:C], op=Op.add)
                nc.vector.reduce_sum(sumexp[:, sl], s1t[:, 0:nt, :], axis=X)

                pending[ui] = (y, sl, nt)
                j = ui - SHIFT
                if j in pending:
                    maxchain(*pending.pop(j))

                if ui == 33:
                    nc.sync.dma_start(warm_in[:], valid_sb[0:1, 0:2])
                    nc.gpsimd.collective_compute(
                        "AllReduce", Op.max,
                        replica_groups=[list(range(N_CORES))],
                        ins=[warm_in[:].opt()], outs=[warm_out[:].opt()])

                # remainder-column pad strips (unc is monotone in sumexp)
                if last:
                    nc.scalar.activation(padb[:], valid_sb[:, MCOLS:COLS],
                                         Act.Copy, bias=1e9, scale=-1e9)
                    nc.vector.tensor_add(se_rem[:, 0:1],
                                         sumexp[:, MCOLS:COLS], padb[:])
                    nc.vector.tensor_sub(se_rem[:, 1:2],
                                         sumexp[:, MCOLS:COLS], padb[:])

            # ---- phase 1b: global umin/umax collective ----
            mm = persist.tile([P, 2], f32)
            mhi = persist.tile([P, 1], f32)
            nc.vector.reduce_max(mhi[:], sumexp[:, 0:MCOLS], axis=X)
            nc.vector.tensor_tensor(mm[:, 0:1], mhi[:], se_rem[:, 1:2],
                                    op=Op.max)
            run_lo = persist.tile([P, 1], f32)
            nc.vector.tensor_reduce(run_lo[:], sumexp[:, 0:MCOLS],
                                    axis=X, op=Op.min)
            nc.vector.tensor_tensor(run_lo[:], run_lo[:], se_rem[:, 0:1],
                                    op=Op.min)
            nc.vector.tensor_scalar(mm[:, 1:2], run_lo[:], -1.0, None,
                                    Op.mult)
            mmr = persist.tile([P, 2], f32)
            nc.gpsimd.partition_all_reduce(mmr[:], mm[:], channels=P,
                                           reduce_op=bass_rust.ReduceOp.max)
            cc1_in = dram.tile([1, 2], f32)
            cc1_out = dram.tile([1, 2], f32)
            nc.sync.dma_start(cc1_in[:], mmr[0:1, :])
            nc.gpsimd.collective_compute(
                "AllReduce", Op.max,
                replica_groups=[list(range(N_CORES))],
                ins=[cc1_in[:].opt()], outs=[cc1_out[:].opt()])
            gmm = persist.tile([P, 2], f32)
            nc.sync.dma_start(
                gmm[:],
                bass.AP(tensor=cc1_out.tensor, offset=cc1_out[:].offset,
                        ap=[[0, P], [1, 2]]))

            # trailing max chains fill the collective's latency window
            for j in sorted(pending):
                maxchain(*pending.pop(j))

            # ---- per-row weights (independent of the collective) ----
            yt = persist.tile([P, CP], bf16)
            nc.scalar.activation(yt[:], xt_sb[:], Act.Exp)
            corr = persist.tile([P, CP], bf16)
            nc.vector.tensor_tensor(corr[:], pmax[:], yt[:], op=Op.is_equal)
            valid_b = persist.tile([P, CP], bf16)
            nc.vector.tensor_copy(valid_b[:], valid_sb[:])

            m1 = persist.tile([P, CP], bf16)             # correct: max_alpha
            nc.vector.scalar_tensor_tensor(m1[:], pmax[:], 1.0, corr[:],
                                           op0=Op.add, op1=Op.mult)
            cmv = persist.tile([P, CP], bf16)
            nc.vector.tensor_sub(cmv[:], corr[:], valid_b[:])
            m0 = persist.tile([P, CP], bf16)             # incorrect: 1-max_a
            nc.vector.tensor_mul(m0[:], cmv[:], pmax[:])

            sumalpha = persist.tile([P, CP], f32)
            nc.scalar.activation(sumalpha[:], sumexp[:], Act.Identity,
                                 bias=c100[:])
            rcp = persist.tile([P, CP], f32)             # 1 / (C + sumexp)
            nc.vector.reciprocal(rcp[:], sumalpha[:])
            t_ = persist.tile([P, CP], bf16)             # tanh(C * rcp)
            nc.scalar.activation(t_[:], rcp[:], Act.Tanh, scale=float(C))
            omt = persist.tile([P, CP], bf16)            # 1 - t
            nc.scalar.activation(omt[:], t_[:], Act.Copy, bias=1.0,
                                 scale=-1.0)

            w4 = persist.tile([P, CP, 4], bf16)          # c-major weights
            nc.vector.tensor_mul(w4[:, :, 0], m1[:], omt[:])   # ac
            nc.vector.tensor_mul(w4[:, :, 1], m1[:], t_[:])    # au
            nc.vector.tensor_mul(w4[:, :, 2], m0[:], omt[:])   # ic
            nc.vector.tensor_mul(w4[:, :, 3], m0[:], t_[:])    # iu
            nc.vector.memset(w4[:, COLS:CP, :], 0.0)

            # gmm holds (max_se, -min_se); umax = C/(C+min_se),
            # umin = C/(C+max_se)
            gsa = persist.tile([P, 2], f32)   # (C+max_se, C+min_se)
            nc.vector.tensor_scalar(gsa[:, 0:1], gmm[:, 0:1], float(C), None,
                                    Op.add)
            nc.vector.tensor_scalar(gsa[:, 1:2], gmm[:, 1:2], -1.0, float(C),
                                    Op.mult, Op.add)
            gu = persist.tile([P, 2], f32)    # (umin, umax)
            nc.vector.reciprocal(gu[:], gsa[:])
            nc.vector.tensor_scalar(gu[:], gu[:], float(C), None, Op.mult)
            # bucket b = (unc - umin) * 20 / (umax - umin)
            #          = rcp * (20*C/rng) - umin*20/rng     (unc = C*rcp)
            rng = persist.tile([P, 1], f32)
            nc.vector.tensor_sub(rng[:], gu[:, 1:2], gu[:, 0:1])
            rrng = persist.tile([P, 1], f32)
            nc.vector.reciprocal(rrng[:], rng[:])
            s1v = persist.tile([P, 1], f32)   # 20*C/rng
            nc.vector.tensor_scalar(s1v[:], rrng[:], float((NTH - 1) * C),
                                    None, Op.mult)
            u0s = persist.tile([P, 1], f32)   # umin*20/rng
            nc.vector.tensor_scalar(u0s[:], rrng[:], gu[:, 0:1],
                                    float(NTH - 1), Op.mult, Op.mult)
            bc = persist.tile([P, CP], bf16)
            nc.vector.scalar_tensor_tensor(
                bc[:], rcp[:], s1v[:], u0s[:].broadcast_to((P, CP)),
                op0=Op.mult, op1=Op.subtract)

            # mask[p, g, k, c] = bc[p, g*G+c] <= k: group-major layout keeps
            # the per-k build in 4x mode (innermost [1, G] bf16 writes) while
            # each group's [K, G] block stays contiguous, so the matmul
            # moving AP collapses to one free dim. Matmuls accumulate into a
            # block-diagonal [4G, GK] PSUM: S[c*4+s, k*G+c] += w4*mask.
            S = psump.tile([4 * G, G * K], f32)
            with tc.tile_pool(name="maskp", bufs=2) as maskp:
                NG = CW // G
                for c0 in range(0, CP, CW):
                    mask = maskp.tile([P, NG, K, G], bf16, tag="mask")
                    bcv = bc[:, c0:c0 + CW].rearrange(
                        "p (g c) -> p g c", c=G)
                    for k in range(K):
                        nc.vector.tensor_scalar(
                            mask[:, :, k, :], bcv, float(k), None, Op.is_le)
                    for gi in range(NG):
                        g0 = c0 + gi * G
                        nc.tensor.matmul(
                            S[:],
                            w4[:, g0:g0 + G, :],
                            mask[:, gi, :, :],
                            start=(g0 == 0), stop=(g0 + G >= CP))

            s_all = persist.tile([4 * G, G * K], f32)
            nc.vector.tensor_copy(s_all[:], S[:])
            # diagonal extraction: n_part[s, c, k] = S[c*4+s, k*G+c]
            s_cat = persist.tile([4, G, K], f32)
            for g in range(G):
                src = s_all[4 * g:4 * g + 4, g:g + 1 + (K - 1) * G]
                nc.sync.dma_start(
                    s_cat[:, g, :],
                    bass.AP(tensor=src.tensor, offset=src.offset,
                            ap=[list(src.ap[0]), [G, K]]))
            s_sb = persist.tile([4, K], f32)
            nc.vector.tensor_reduce(
                s_sb[:],
                bass.AP(tensor=s_cat.tensor, offset=s_cat[:].offset,
                        ap=[list(s_cat[:].ap[0]), [1, K], [K, G]]),
                axis=X, op=Op.add)
            cc2_in = dram.tile([4, K], f32)
            cc2_out = dram.tile([4, K], f32)
            nc.sync.dma_start(cc2_in[:], s_sb[:])
            nc.gpsimd.collective_compute(
                "AllReduce", Op.add,
                replica_groups=[list(range(N_CORES))],
                ins=[cc2_in[:].opt()], outs=[cc2_out[:].opt()])

            # flatten [4, K] -> [1, 4K] on partition 0
            f = persist.tile([1, 4 * K], f32)
            nc.sync.dma_start(
                f[:], cc2_out[:].rearrange("a b -> (a b)").unsqueeze(0))

            ac = f[:, 0:NTH]
            aup = f[:, K:K + NTH]
            t_au = f[:, K + NTH:K + NTH + 1]
            ic = f[:, 2 * K:2 * K + NTH]
            iup = f[:, 3 * K:3 * K + NTH]
            t_iu = f[:, 3 * K + NTH:3 * K + NTH + 1]

            nneg_iu = persist.tile([1, NTH], f32)        # -n_iu
            nc.vector.tensor_scalar(nneg_iu[:], iup, t_iu, None, Op.subtract)
            num = persist.tile([1, NTH], f32)            # n_ac + n_iu
            nc.vector.tensor_sub(num[:], ac, nneg_iu[:])
            nneg_au = persist.tile([1, NTH], f32)        # -n_au
            nc.vector.tensor_scalar(nneg_au[:], aup, t_au, None, Op.subtract)
            den = persist.tile([1, NTH], f32)
            nc.vector.tensor_sub(den[:], num[:], nneg_au[:])
            nc.vector.tensor_add(den[:], den[:], ic)
            nc.vector.tensor_scalar(den[:], den[:], EPS, None, Op.add)
            rden = persist.tile([1, NTH], f32)
            nc.vector.reciprocal(rden[:], den[:])
            evu = persist.tile([1, NTH], f32)
            nc.vector.tensor_mul(evu[:], num[:], rden[:])

            ssum = persist.tile([1, 1], f32)
            nc.vector.reduce_sum(ssum[:], evu[:], axis=X)
            edge = persist.tile([1, 1], f32)
            nc.vector.tensor_add(edge[:], evu[:, 0:1], evu[:, NTH - 1:NTH])
            nc.vector.tensor_scalar(edge[:], edge[:], 0.5, None, Op.mult)
            auc = persist.tile([1, 1], f32)
            nc.vector.tensor_sub(auc[:], ssum[:], edge[:])
            nc.vector.tensor_scalar(auc[:], auc[:], 1.0 / (NTH - 1), None,
                                    Op.mult)
            eps_t = persist.tile([1, 1], f32)
            nc.vector.memset(eps_t[:], EPS)
            nll = persist.tile([1, 1], f32)
            nc.scalar.activation(nll[:], auc[:], Act.Ln, bias=eps_t[:])
            res = persist.tile([1, 1], f32)
            nc.vector.tensor_scalar(res[:], nll[:], -1.0, None, Op.mult)
            nc.sync.dma_start(out_d.ap(), res[:])

    nc.compile()
    return nc


_NC = None


def _get_nc():
    global _NC
    if _NC is None:
        _NC = _build_nc()
    return _NC


_VALID = None


def _valid_mask():
    global _VALID
    if _VALID is None:
        v = np.ones((P, CP), np.float32)
        v[:, MCOLS:] = 0.0
        v[:REM, MCOLS] = 1.0
        _VALID = v
    return _VALID


def _in_maps(output, target):
    output = np.ascontiguousarray(np.asarray(output, dtype=np.float32))
    tgt = np.asarray(target).astype(np.int64)
    xt_full = output[np.arange(output.shape[0]), tgt].astype(np.float32)
    v = _valid_mask()
    maps = []
    for i in range(N_CORES):
        xs = output[i * NPC:(i + 1) * NPC]
        xtc = xt_full[i * NPC:(i + 1) * NPC]
        xtm = np.full((P, CP), -1e30, np.float32)
        xtm[:, :MCOLS] = xtc[:P * MCOLS].reshape(P, MCOLS)
        xtm[:REM, MCOLS] = xtc[P * MCOLS:]
        maps.append({"x": xs, "xt": xtm, "valid": v})
    return maps


def run(output, target, trace=False):
    from concourse.bass_utils import run_bass_kernel_spmd
    nc = _get_nc()
    res = run_bass_kernel_spmd(nc, _in_maps(output, target),
                               core_ids=list(range(N_CORES)), trace=trace)
    val = np.float32(res.results[0]["out"][0, 0])
    return val, res


def kernel(output, target, num_classes):
    assert int(num_classes) == C
    val, _ = run(output, target)
    return np.array(val, dtype=np.float32)


# revision 9
# speedup vs baseline: 1.1440x; 1.1440x over previous
"""Trainium2 Bass kernel for nn_A2EvULoss (EvU loss over [1M, 100] logits).

Data-parallel over 8 NeuronCores; each core streams its 125k-row shard once
from HBM (p-major layout: partition p holds rows p*976+c, giving 6.4KB
contiguous DMA bursts per partition).

Streaming phase (per 16-row-tile chunk), balanced so the DMA is the wall:
 - ScalarE: one batched exp over the whole chunk, output in bf16 (~1.5us).
 - GpSimd: pairwise add of the two class halves (bf16 in, f32 out) (~1.8us).
 - DVE: segmented reduce_sum of the pairwise sums gives per-row sumexp;
   a bf16 max chain (two 2x tensor_tensor max levels, the second using an
   overlapped split since max is idempotent, then a short reduce_max)
   gives the per-row max evidence exp(xmax) directly (~2.1us).
Max chains run SHIFT chunks behind the stream so the last few fill the
latency window of the umin/umax all-reduce after the stream ends.

Correctness test uses a host-side gather: xt[i] = x[i, target[i]] is fed
as a second (tiny) input; the row is correct iff bf16(exp(xt)) equals the
bf16 max of exp(x) (identical spline + rounding path, so equality is
exact for true argmax rows; bf16-tie false positives are ~7e-5 of rows).

Tail: all-reduce (max) of (max sumexp, -min sumexp) gives global umin/umax
(unc = C/(C+sumexp) is monotone); per-row weights go to four bf16 planes
(c-major [128, 992, 4]); 22 threshold masks are built per column chunk with
4x-mode tensor_scalar is_le against a bf16 bucket index in a group-major
layout; 62 PSUM-accumulated matmuls (G=16 stat columns each) produce a
block-diagonal [64, 352] PSUM; strided DMAs extract the [4, 16, 22]
diagonal, a reduce gives [4, 22], which is all-reduced; every core computes
the trapezoid AUC + -log replicated; core 0's scalar is returned.
"""

import numpy as np

P = 128
C = 100
H = C // 2                        # 50: pairwise split of the class dim
N_CORES = 8
N_TOTAL = 1_000_000
NPC = N_TOTAL // N_CORES          # 125000 rows per core
MCOLS = NPC // P                  # 976 main stat columns (p-major layout)
REM = NPC - P * MCOLS             # 72 remainder rows
COLS = MCOLS + 1                  # 977 stat columns (col 976 = remainder)
CP = 992                          # padded to a multiple of G=16 for matmuls
T = 16                            # row-tiles per streaming chunk
NCHUNKS = MCOLS // T              # 61
SHIFT = 5                         # max chains run SHIFT units behind stream
NTH = 21
K = NTH + 1                       # 21 thresholds + 1 all-ones (totals) column
G = 16                            # stat columns per matmul group
CW = 496                          # mask-build chunk width (31 groups; 2*496)
EPS = 1e-10


def _build_nc():
    import bass_rust
    import concourse.bass as bass
    import concourse.bacc as bacc
    import concourse.tile as tile
    from concourse import mybir

    f32 = mybir.dt.float32
    bf16 = mybir.dt.bfloat16
    Op = mybir.AluOpType
    Act = mybir.ActivationFunctionType
    X = mybir.AxisListType.X

    nc = bacc.Bacc("TRN2", target_bir_lowering=False, debug=False,
                   num_devices=N_CORES)

    x_d = nc.dram_tensor("x", [NPC, C], f32, kind="ExternalInput")
    xt_d = nc.dram_tensor("xt", [P, CP], f32, kind="ExternalInput")
    valid_d = nc.dram_tensor("valid", [P, CP], f32, kind="ExternalInput")
    out_d = nc.dram_tensor("out", [1, 1], f32, kind="ExternalOutput")

    x_main = x_d.ap()[0:P * MCOLS, :].rearrange("(p c) f -> p (c f)", p=P)
    x_rem = x_d.ap()[P * MCOLS:NPC, :]                      # [72, 100]

    with tile.TileContext(nc) as tc:
        with (
            tc.tile_pool(name="stream", bufs=3) as stream,
            tc.tile_pool(name="ystream", bufs=SHIFT + 2) as ystream,
            tc.tile_pool(name="persist", bufs=1) as persist,
            tc.tile_pool(name="psum", bufs=1, space="PSUM") as psump,
            tc.tile_pool(name="dram", bufs=1, space="DRAM") as dram,
        ):
            # ---- persistent inputs / stats ----
            xt_sb = persist.tile([P, CP], f32)
            nc.sync.dma_start(xt_sb[:], xt_d.ap())
            valid_sb = persist.tile([P, CP], f32)
            nc.sync.dma_start(valid_sb[:], valid_d.ap())

            pmax = persist.tile([P, CP], bf16)     # per-row max of exp(x)
            sumexp = persist.tile([P, CP], f32)
            nc.vector.memset(pmax[:, COLS:CP], 0.0)
            nc.vector.memset(sumexp[:, COLS:CP], 0.0)
            padb = persist.tile([P, 1], f32)
            se_rem = persist.tile([P, 2], f32)  # (min-in, max-in) of col 976

            c100 = persist.tile([P, 1], f32)
            nc.vector.memset(c100[:], float(C))

            # warm-up/sync collective: wakes the CC path and re-syncs core
            # skew mid-stream so the real all-reduces wait less
            warm_in = dram.tile([1, 2], f32)
            warm_out = dram.tile([1, 2], f32)

            # ---- phase 1: stream x, compute y=exp(x) bf16, max + sumexp ----
            # zero the two pad columns of each s1 buffer once; the stream
            # only ever writes cols 0:50, so the 26+26 split of 52 stays
            # exact (pads contribute 0 to the pairwise sums)
            for _ in range(3):
                s1w = stream.tile([P, T, H + 2], bf16, tag="s1")
                nc.vector.memset(s1w[:, :, H:H + 2], 0.0)

            def maxchain(y, sl, nt):
                yv = y[:, 0:nt * C].rearrange("p (t f) -> p t f", f=C)
                mx1 = stream.tile([P, T, H], bf16, tag="mx1")
                nc.vector.tensor_tensor(
                    mx1[:, 0:nt, :], yv[:, :, 0:H], yv[:, :, H:C], op=Op.max)
                # overlapped split (24:50 vs 0:26) keeps both operands
                # 4B-aligned; max is idempotent so the overlap is harmless
                mx2 = stream.tile([P, T, 26], bf16, tag="mx2")
                nc.vector.tensor_tensor(
                    mx2[:, 0:nt, :], mx1[:, 0:nt, 0:26], mx1[:, 0:nt, 24:H],
                    op=Op.max)
                nc.vector.reduce_max(pmax[:, sl], mx2[:, 0:nt, :], axis=X)

            # chunk 0 split into 4 small sub-chunks so compute starts as
            # soon as the first 200KB lands
            units = [(4 * i, 4, False) for i in range(4)]
            units += [(16 + T * i, T, False) for i in range(NCHUNKS - 1)]
            units += [(MCOLS, 1, True)]
            pending = {}
            for ui, (c0u, nt, last) in enumerate(units):
                xtile = stream.tile([P, T * C], f32, tag="xt")
                if last:
                    nc.vector.memset(xtile[:, 0:C], 0.0)
                    nc.sync.dma_start(xtile[0:REM, 0:C], x_rem)
                    sl = slice(MCOLS, COLS)
                else:
                    nc.sync.dma_start(
                        xtile[:, 0:nt * C],
                        x_main[:, c0u * C:(c0u + nt) * C])
                    sl = slice(c0u, c0u + nt)

                y = ystream.tile([P, T * C], bf16, tag="y")
                nc.scalar.activation(y[:, 0:nt * C], xtile[:, 0:nt * C],
                                     Act.Exp)

                yv = y[:, 0:nt * C].rearrange("p (t f) -> p t f", f=C)
                s1t = stream.tile([P, T, H + 2], bf16, tag="s1")
                nc.vector.tensor_tensor(
                    s1t[:, 0:nt, 0:H], yv[:, :, 0:H], yv[:, :, H:C],
                    op=Op.add)
                s2t = stream.tile([P, T, 26], bf16, tag="s2")
                nc.vector.tensor_tensor(
                    s2t[:, 0:nt, :], s1t[:, 0:nt, 0:26], s1t[:, 0:nt, 26:52],
                    op=Op.add)
                nc.vector.reduce_sum(sumexp[:, sl], s2t[:, 0:nt, :], axis=X)

                pending[ui] = (y, sl, nt)
                j = ui - SHIFT
                if j in pending:
                    maxchain(*pending.pop(j))

                if ui == 33:
                    nc.sync.dma_start(warm_in[:], valid_sb[0:1, 0:2])
                    nc.gpsimd.collective_compute(
                        "AllReduce", Op.max,
                        replica_groups=[list(range(N_CORES))],
                        ins=[warm_in[:].opt()], outs=[warm_out[:].opt()])

                # remainder-column pad strips (unc is monotone in sumexp)
                if last:
                    nc.scalar.activation(padb[:], valid_sb[:, MCOLS:COLS],
                                         Act.Copy, bias=1e9, scale=-1e9)
                    nc.vector.tensor_add(se_rem[:, 0:1],
                                         sumexp[:, MCOLS:COLS], padb[:])
                    nc.vector.tensor_sub(se_rem[:, 1:2],
                                         sumexp[:, MCOLS:COLS], padb[:])

            # ---- phase 1b: global umin/umax collective ----
            mm = persist.tile([P, 2], f32)
            mhi = persist.tile([P, 1], f32)
            nc.vector.reduce_max(mhi[:], sumexp[:, 0:MCOLS], axis=X)
            nc.vector.tensor_tensor(mm[:, 0:1], mhi[:], se_rem[:, 1:2],
                                    op=Op.max)
            run_lo = persist.tile([P, 1], f32)
            nc.vector.tensor_reduce(run_lo[:], sumexp[:, 0:MCOLS],
                                    axis=X, op=Op.min)
            nc.vector.tensor_tensor(run_lo[:], run_lo[:], se_rem[:, 0:1],
                                    op=Op.min)
            nc.vector.tensor_scalar(mm[:, 1:2], run_lo[:], -1.0, None,
                                    Op.mult)
            mmr = persist.tile([P, 2], f32)
            nc.gpsimd.partition_all_reduce(mmr[:], mm[:], channels=P,
                                           reduce_op=bass_rust.ReduceOp.max)
            cc1_in = dram.tile([1, 2], f32)
            cc1_out = dram.tile([1, 2], f32)
            nc.sync.dma_start(cc1_in[:], mmr[0:1, :])
            nc.gpsimd.collective_compute(
                "AllReduce", Op.max,
                replica_groups=[list(range(N_CORES))],
                ins=[cc1_in[:].opt()], outs=[cc1_out[:].opt()])
            gmm = persist.tile([P, 2], f32)
            nc.sync.dma_start(
                gmm[:],
                bass.AP(tensor=cc1_out.tensor, offset=cc1_out[:].offset,
                        ap=[[0, P], [1, 2]]))

            # trailing max chains fill the collective's latency window
            for j in sorted(pending):
                maxchain(*pending.pop(j))

            # ---- per-row weights (independent of the collective) ----
            yt = persist.tile([P, CP], bf16)
            nc.scalar.activation(yt[:], xt_sb[:], Act.Exp)
            corr = persist.tile([P, CP], bf16)
            nc.vector.tensor_tensor(corr[:], pmax[:], yt[:], op=Op.is_equal)
            valid_b = persist.tile([P, CP], bf16)
            nc.vector.tensor_copy(valid_b[:], valid_sb[:])

            m1 = persist.tile([P, CP], bf16)             # correct: max_alpha
            nc.vector.scalar_tensor_tensor(m1[:], pmax[:], 1.0, corr[:],
                                           op0=Op.add, op1=Op.mult)
            cmv = persist.tile([P, CP], bf16)
            nc.vector.tensor_sub(cmv[:], corr[:], valid_b[:])
            m0 = persist.tile([P, CP], bf16)             # incorrect: 1-max_a
            nc.vector.tensor_mul(m0[:], cmv[:], pmax[:])

            sumalpha = persist.tile([P, CP], f32)
            nc.scalar.activation(sumalpha[:], sumexp[:], Act.Identity,
                                 bias=c100[:])
            rcp = persist.tile([P, CP], f32)             # 1 / (C + sumexp)
            nc.vector.reciprocal(rcp[:], sumalpha[:])
            t_ = persist.tile([P, CP], bf16)             # tanh(C * rcp)
            nc.scalar.activation(t_[:], rcp[:], Act.Tanh, scale=float(C))
            omt = persist.tile([P, CP], bf16)            # 1 - t
            nc.scalar.activation(omt[:], t_[:], Act.Copy, bias=1.0,
                                 scale=-1.0)

            w4 = persist.tile([P, CP, 4], bf16)          # c-major weights
            nc.vector.tensor_mul(w4[:, :, 0], m1[:], omt[:])   # ac
            nc.vector.tensor_mul(w4[:, :, 1], m1[:], t_[:])    # au
            nc.vector.tensor_mul(w4[:, :, 2], m0[:], omt[:])   # ic
            nc.vector.tensor_mul(w4[:, :, 3], m0[:], t_[:])    # iu
            nc.vector.memset(w4[:, COLS:CP, :], 0.0)

            # gmm holds (max_se, -min_se); umax = C/(C+min_se),
            # umin = C/(C+max_se)
            gsa = persist.tile([P, 2], f32)   # (C+max_se, C+min_se)
            nc.vector.tensor_scalar(gsa[:, 0:1], gmm[:, 0:1], float(C), None,
                                    Op.add)
            nc.vector.tensor_scalar(gsa[:, 1:2], gmm[:, 1:2], -1.0, float(C),
                                    Op.mult, Op.add)
            gu = persist.tile([P, 2], f32)    # (umin, umax)
            nc.vector.reciprocal(gu[:], gsa[:])
            nc.vector.tensor_scalar(gu[:], gu[:], float(C), None, Op.mult)
            # bucket b = (unc - umin) * 20 / (umax - umin)
            #          = rcp * (20*C/rng) - umin*20/rng     (unc = C*rcp)
            rng = persist.tile([P, 1], f32)
            nc.vector.tensor_sub(rng[:], gu[:, 1:2], gu[:, 0:1])
            rrng = persist.tile([P, 1], f32)
            nc.vector.reciprocal(rrng[:], rng[:])
            s1v = persist.tile([P, 1], f32)   # 20*C/rng
            nc.vector.tensor_scalar(s1v[:], rrng[:], float((NTH - 1) * C),
                                    None, Op.mult)
            u0s = persist.tile([P, 1], f32)   # umin*20/rng
            nc.vector.tensor_scalar(u0s[:], rrng[:], gu[:, 0:1],
                                    float(NTH - 1), Op.mult, Op.mult)
            bc = persist.tile([P, CP], bf16)
            nc.vector.scalar_tensor_tensor(
                bc[:], rcp[:], s1v[:], u0s[:].broadcast_to((P, CP)),
                op0=Op.mult, op1=Op.subtract)

            # mask[p, g, k, c] = bc[p, g*G+c] <= k: group-major layout keeps
            # the per-k build in 4x mode (innermost [1, G] bf16 writes) while
            # each group's [K, G] block stays contiguous, so the matmul
            # moving AP collapses to one free dim. Matmuls accumulate into a
            # block-diagonal [4G, GK] PSUM: S[c*4+s, k*G+c] += w4*mask.
            S = psump.tile([4 * G, G * K], f32)
            with tc.tile_pool(name="maskp", bufs=2) as maskp:
                NG = CW // G
                for c0 in range(0, CP, CW):
                    mask = maskp.tile([P, NG, K, G], bf16, tag="mask")
                    bcv = bc[:, c0:c0 + CW].rearrange(
                        "p (g c) -> p g c", c=G)
                    for k in range(K):
                        nc.vector.tensor_scalar(
                            mask[:, :, k, :], bcv, float(k), None, Op.is_le)
                    for gi in range(NG):
                        g0 = c0 + gi * G
                        nc.tensor.matmul(
                            S[:],
                            w4[:, g0:g0 + G, :],
                            mask[:, gi, :, :],
                            start=(g0 == 0), stop=(g0 + G >= CP))

            s_all = persist.tile([4 * G, G * K], f32)
            nc.vector.tensor_copy(s_all[:], S[:])
            # diagonal extraction: n_part[s, c, k] = S[c*4+s, k*G+c].
            # Round-trip S through flat DRAM; the reload's partition stride
            # of 16*GK+4 bakes the per-c-quarter diagonal shift in, so one
            # uniform strided reduce yields the [4, s*K] partials.
            ROW = G * K                                    # 352
            QS = (4 * G // 4) * ROW + 4                    # 5636
            dflat = dram.tile([1, 4 * G * ROW + 16], f32)
            nc.sync.dma_start(
                bass.AP(tensor=dflat.tensor, offset=dflat[:].offset,
                        ap=[[ROW, 4 * G], [1, ROW]]),
                s_all[:])
            s_line = persist.tile([4, (4 * G // 4) * ROW], f32)
            nc.sync.dma_start(
                s_line[:],
                bass.AP(tensor=dflat.tensor, offset=dflat[:].offset,
                        ap=[[QS, 4], [1, (4 * G // 4) * ROW]]))
            s_sb = persist.tile([4, 4 * K], f32)   # per-quarter [s, k] sums
            nc.vector.tensor_reduce(
                s_sb[:],
                bass.AP(tensor=s_line.tensor, offset=s_line[:].offset,
                        ap=[list(s_line[:].ap[0]), [ROW, 4], [G, K],
                            [4 * ROW + 1, 4]]),
                axis=X, op=Op.add)
            cc2_in = dram.tile([4, 4 * K], f32)
            cc2_out = dram.tile([4, 4 * K], f32)
            nc.sync.dma_start(cc2_in[:], s_sb[:])
            nc.gpsimd.collective_compute(
                "AllReduce", Op.add,
                replica_groups=[list(range(N_CORES))],
                ins=[cc2_in[:].opt()], outs=[cc2_out[:].opt()])

            # flatten [4, 4K] -> [1, 16K] on partition 0, sum the quarters
            fcat = persist.tile([1, 16 * K], f32)
            nc.sync.dma_start(
                fcat[:], cc2_out[:].rearrange("a b -> (a b)").unsqueeze(0))
            f = persist.tile([1, 4 * K], f32)
            nc.vector.tensor_reduce(
                f[:],
                bass.AP(tensor=fcat.tensor, offset=fcat[:].offset,
                        ap=[list(fcat[:].ap[0]), [1, 4 * K], [4 * K, 4]]),
                axis=X, op=Op.add)

            ac = f[:, 0:NTH]
            aup = f[:, K:K + NTH]
            t_au = f[:, K + NTH:K + NTH + 1]
            ic = f[:, 2 * K:2 * K + NTH]
            iup = f[:, 3 * K:3 * K + NTH]
            t_iu = f[:, 3 * K + NTH:3 * K + NTH + 1]

            nneg_iu = persist.tile([1, NTH], f32)        # -n_iu
            nc.vector.tensor_scalar(nneg_iu[:], iup, t_iu, None, Op.subtract)
            num = persist.tile([1, NTH], f32)            # n_ac + n_iu
            nc.vector.tensor_sub(num[:], ac, nneg_iu[:])
            nneg_au = persist.tile([1, NTH], f32)        # -n_au
            nc.vector.tensor_scalar(nneg_au[:], aup, t_au, None, Op.subtract)
            den = persist.tile([1, NTH], f32)
            nc.vector.tensor_sub(den[:], num[:], nneg_au[:])
            nc.vector.tensor_add(den[:], den[:], ic)
            nc.vector.tensor_scalar(den[:], den[:], EPS, None, Op.add)
            rden = persist.tile([1, NTH], f32)
            nc.vector.reciprocal(rden[:], den[:])
            evu = persist.tile([1, NTH], f32)
            nc.vector.tensor_mul(evu[:], num[:], rden[:])

            ssum = persist.tile([1, 1], f32)
            nc.vector.reduce_sum(ssum[:], evu[:], axis=X)
            edge = persist.tile([1, 1], f32)
            nc.vector.tensor_add(edge[:], evu[:, 0:1], evu[:, NTH - 1:NTH])
            nc.vector.tensor_scalar(edge[:], edge[:], 0.5, None, Op.mult)
            auc = persist.tile([1, 1], f32)
            nc.vector.tensor_sub(auc[:], ssum[:], edge[:])
            nc.vector.tensor_scalar(auc[:], auc[:], 1.0 / (NTH - 1), None,
                                    Op.mult)
            eps_t = persist.tile([1, 1], f32)
            nc.vector.memset(eps_t[:], EPS)
            nll = persist.tile([1, 1], f32)
            nc.scalar.activation(nll[:], auc[:], Act.Ln, bias=eps_t[:])
            res = persist.tile([1, 1], f32)
            nc.vector.tensor_scalar(res[:], nll[:], -1.0, None, Op.mult)
            nc.sync.dma_start(out_d.ap(), res[:])

    nc.compile()
    return nc


_NC = None


def _get_nc():
    global _NC
    if _NC is None:
        _NC = _build_nc()
    return _NC


_VALID = None


def _valid_mask():
    global _VALID
    if _VALID is None:
        v = np.ones((P, CP), np.float32)
        v[:, MCOLS:] = 0.0
        v[:REM, MCOLS] = 1.0
        _VALID = v
    return _VALID


def _in_maps(output, target):
    output = np.ascontiguousarray(np.asarray(output, dtype=np.float32))
    tgt = np.asarray(target).astype(np.int64)
    xt_full = output[np.arange(output.shape[0]), tgt].astype(np.float32)
    v = _valid_mask()
    maps = []
    for i in range(N_CORES):
        xs = output[i * NPC:(i + 1) * NPC]
        xtc = xt_full[i * NPC:(i + 1) * NPC]
        xtm = np.full((P, CP), -1e30, np.float32)
        xtm[:, :MCOLS] = xtc[:P * MCOLS].reshape(P, MCOLS)
        xtm[:REM, MCOLS] = xtc[P * MCOLS:]
        maps.append({"x": xs, "xt": xtm, "valid": v})
    return maps


def run(output, target, trace=False):
    from concourse.bass_utils import run_bass_kernel_spmd
    nc = _get_nc()
    res = run_bass_kernel_spmd(nc, _in_maps(output, target),
                               core_ids=list(range(N_CORES)), trace=trace)
    val = np.float32(res.results[0]["out"][0, 0])
    return val, res


def kernel(output, target, num_classes):
    assert int(num_classes) == C
    val, _ = run(output, target)
    return np.array(val, dtype=np.float32)


# revision 15
# speedup vs baseline: 1.2151x; 1.0622x over previous
"""Trainium2 Bass kernel for nn_A2EvULoss (EvU loss over [1M, 100] logits).

Data-parallel over 8 NeuronCores; each core streams its 125k-row shard once
from HBM (p-major layout: partition p holds rows p*976+c, giving 6.4KB
contiguous DMA bursts per partition).

Streaming phase (per 16-row-tile chunk), balanced so the DMA is the wall:
 - ScalarE: one batched exp over the whole chunk, output in bf16 (~1.5us).
 - GpSimd: pairwise add of the two class halves (bf16 in, f32 out) (~1.8us).
 - DVE: segmented reduce_sum of the pairwise sums gives per-row sumexp;
   a bf16 max chain (two 2x tensor_tensor max levels, the second using an
   overlapped split since max is idempotent, then a short reduce_max)
   gives the per-row max evidence exp(xmax) directly (~2.1us).
Max chains run SHIFT chunks behind the stream so the last few fill the
latency window of the umin/umax all-reduce after the stream ends.

Correctness test uses a host-side gather: xt[i] = x[i, target[i]] is fed
as a second (tiny) input; the row is correct iff bf16(exp(xt)) equals the
bf16 max of exp(x) (identical spline + rounding path, so equality is
exact for true argmax rows; bf16-tie false positives are ~7e-5 of rows).

Tail: all-reduce (max) of (max sumexp, -min sumexp) gives global umin/umax
(unc = C/(C+sumexp) is monotone); per-row weights go to four bf16 planes
(c-major [128, 992, 4]); 22 threshold masks are built per column chunk with
4x-mode tensor_scalar is_le against a bf16 bucket index in a group-major
layout; 62 PSUM-accumulated matmuls (G=16 stat columns each) produce a
block-diagonal [64, 352] PSUM; strided DMAs extract the [4, 16, 22]
diagonal, a reduce gives [4, 22], which is all-reduced; every core computes
the trapezoid AUC + -log replicated; core 0's scalar is returned.
"""

import numpy as np

P = 128
C = 100
H = C // 2                        # 50: pairwise split of the class dim
N_CORES = 8
N_TOTAL = 1_000_000
NPC = N_TOTAL // N_CORES          # 125000 rows per core
MCOLS = NPC // P                  # 976 main stat columns (p-major layout)
REM = NPC - P * MCOLS             # 72 remainder rows
COLS = MCOLS + 1                  # 977 stat columns (col 976 = remainder)
CP = 992                          # padded to a multiple of G=16 for matmuls
T = 32                            # row-tiles per streaming chunk
NCHUNKS = (MCOLS - 16) // T       # 30 full chunks after the 2 warm-up units
SHIFT = 3                         # max chains run SHIFT units behind stream
NTH = 21
K = NTH + 1                       # 21 thresholds + 1 all-ones (totals) column
G = 16                            # stat columns per matmul group
CWS = (336, 336, 320)             # mask-build chunk widths (21+21+20 groups)
EPS = 1e-10


def _build_nc():
    import bass_rust
    import concourse.bass as bass
    import concourse.bacc as bacc
    import concourse.tile as tile
    from concourse import mybir

    f32 = mybir.dt.float32
    bf16 = mybir.dt.bfloat16
    Op = mybir.AluOpType
    Act = mybir.ActivationFunctionType
    X = mybir.AxisListType.X

    nc = bacc.Bacc("TRN2", target_bir_lowering=False, debug=False,
                   num_devices=N_CORES)

    x_d = nc.dram_tensor("x", [NPC, C], f32, kind="ExternalInput")
    xt_d = nc.dram_tensor("xt", [P, CP], f32, kind="ExternalInput")
    valid_d = nc.dram_tensor("valid", [P, CP], f32, kind="ExternalInput")
    out_d = nc.dram_tensor("out", [1, 1], f32, kind="ExternalOutput")

    x_main = x_d.ap()[0:P * MCOLS, :].rearrange("(p c) f -> p (c f)", p=P)
    x_rem = x_d.ap()[P * MCOLS:NPC, :]                      # [72, 100]

    with tile.TileContext(nc) as tc:
        with (
            tc.tile_pool(name="stream", bufs=3) as stream,
            tc.tile_pool(name="ystream", bufs=SHIFT + 1) as ystream,
            tc.tile_pool(name="persist", bufs=1) as persist,
            tc.tile_pool(name="psum", bufs=1, space="PSUM") as psump,
            tc.tile_pool(name="dram", bufs=1, space="DRAM") as dram,
        ):
            # ---- persistent inputs / stats ----
            xt_sb = persist.tile([P, CP], f32)
            nc.sync.dma_start(xt_sb[:], xt_d.ap())
            valid_sb = persist.tile([P, CP], f32)
            nc.sync.dma_start(valid_sb[:], valid_d.ap())

            pmax = persist.tile([P, CP], bf16)     # per-row max of exp(x)
            sumexp = persist.tile([P, CP], f32)
            nc.vector.memset(pmax[:, COLS:CP], 0.0)
            nc.vector.memset(sumexp[:, COLS:CP], 0.0)
            padb = persist.tile([P, 1], f32)
            se_rem = persist.tile([P, 2], f32)  # (min-in, max-in) of col 976

            c100 = persist.tile([P, 1], f32)
            nc.vector.memset(c100[:], float(C))

            # warm-up/sync collective: wakes the CC path and re-syncs core
            # skew mid-stream so the real all-reduces wait less
            warm_in = dram.tile([1, 2], f32)
            warm_out = dram.tile([1, 2], f32)

            # ---- phase 1: stream x, compute y=exp(x) bf16, max + sumexp ----
            # zero the two pad columns of each s1 buffer once; the stream
            # only ever writes cols 0:50, so the 26+26 split of 52 stays
            # exact (pads contribute 0 to the pairwise sums)
            for _ in range(3):
                s1w = stream.tile([P, T, H + 2], bf16, tag="s1")
                nc.vector.memset(s1w[:, :, H:H + 2], 0.0)

            def maxchain(y, sl, nt):
                yv = y[:, 0:nt * C].rearrange("p (t f) -> p t f", f=C)
                mx1 = stream.tile([P, T, H], bf16, tag="mx1")
                nc.vector.tensor_tensor(
                    mx1[:, 0:nt, :], yv[:, :, 0:H], yv[:, :, H:C], op=Op.max)
                # overlapped split (24:50 vs 0:26) keeps both operands
                # 4B-aligned; max is idempotent so the overlap is harmless
                mx2 = stream.tile([P, T, 26], bf16, tag="mx2")
                nc.vector.tensor_tensor(
                    mx2[:, 0:nt, :], mx1[:, 0:nt, 0:26], mx1[:, 0:nt, 24:H],
                    op=Op.max)
                nc.vector.reduce_max(pmax[:, sl], mx2[:, 0:nt, :], axis=X)

            # chunk 0 split into 2 small sub-chunks so compute starts as
            # soon as the first 400KB lands
            units = [(8 * i, 8, False) for i in range(2)]
            units += [(16 + T * i, T, False) for i in range(NCHUNKS)]
            units += [(MCOLS, 1, True)]
            pending = {}
            for ui, (c0u, nt, last) in enumerate(units):
                xtile = stream.tile([P, T * C], f32, tag="xt")
                if last:
                    nc.vector.memset(xtile[:, 0:C], 0.0)
                    nc.sync.dma_start(xtile[0:REM, 0:C], x_rem)
                    sl = slice(MCOLS, COLS)
                else:
                    nc.sync.dma_start(
                        xtile[:, 0:nt * C],
                        x_main[:, c0u * C:(c0u + nt) * C])
                    sl = slice(c0u, c0u + nt)

                y = ystream.tile([P, T * C], bf16, tag="y")
                nc.scalar.activation(y[:, 0:nt * C], xtile[:, 0:nt * C],
                                     Act.Exp)

                yv = y[:, 0:nt * C].rearrange("p (t f) -> p t f", f=C)
                s1t = stream.tile([P, T, H + 2], bf16, tag="s1")
                nc.vector.tensor_tensor(
                    s1t[:, 0:nt, 0:H], yv[:, :, 0:H], yv[:, :, H:C],
                    op=Op.add)
                s2t = stream.tile([P, T, 26], bf16, tag="s2")
                nc.vector.tensor_tensor(
                    s2t[:, 0:nt, :], s1t[:, 0:nt, 0:26], s1t[:, 0:nt, 26:52],
                    op=Op.add)
                nc.vector.reduce_sum(sumexp[:, sl], s2t[:, 0:nt, :], axis=X)

                pending[ui] = (y, sl, nt)
                j = ui - SHIFT
                if j in pending:
                    maxchain(*pending.pop(j))

                if ui == 17:
                    nc.sync.dma_start(warm_in[:], valid_sb[0:1, 0:2])
                    nc.gpsimd.collective_compute(
                        "AllReduce", Op.max,
                        replica_groups=[list(range(N_CORES))],
                        ins=[warm_in[:].opt()], outs=[warm_out[:].opt()])

                # remainder-column pad strips (unc is monotone in sumexp)
                if last:
                    nc.scalar.activation(padb[:], valid_sb[:, MCOLS:COLS],
                                         Act.Copy, bias=1e9, scale=-1e9)
                    nc.vector.tensor_add(se_rem[:, 0:1],
                                         sumexp[:, MCOLS:COLS], padb[:])
                    nc.vector.tensor_sub(se_rem[:, 1:2],
                                         sumexp[:, MCOLS:COLS], padb[:])

            # ---- phase 1b: global umin/umax collective ----
            mm = persist.tile([P, 2], f32)
            mhi = persist.tile([P, 1], f32)
            nc.vector.reduce_max(mhi[:], sumexp[:, 0:MCOLS], axis=X)
            nc.vector.tensor_tensor(mm[:, 0:1], mhi[:], se_rem[:, 1:2],
                                    op=Op.max)
            run_lo = persist.tile([P, 1], f32)
            nc.vector.tensor_reduce(run_lo[:], sumexp[:, 0:MCOLS],
                                    axis=X, op=Op.min)
            nc.vector.tensor_tensor(run_lo[:], run_lo[:], se_rem[:, 0:1],
                                    op=Op.min)
            nc.vector.tensor_scalar(mm[:, 1:2], run_lo[:], -1.0, None,
                                    Op.mult)
            mmr = persist.tile([P, 2], f32)
            nc.gpsimd.partition_all_reduce(mmr[:], mm[:], channels=P,
                                           reduce_op=bass_rust.ReduceOp.max)
            cc1_in = dram.tile([1, 2], f32)
            cc1_out = dram.tile([1, 2], f32)
            nc.sync.dma_start(cc1_in[:], mmr[0:1, :])
            nc.gpsimd.collective_compute(
                "AllReduce", Op.max,
                replica_groups=[list(range(N_CORES))],
                ins=[cc1_in[:].opt()], outs=[cc1_out[:].opt()])
            gmm = persist.tile([P, 2], f32)
            nc.sync.dma_start(
                gmm[:],
                bass.AP(tensor=cc1_out.tensor, offset=cc1_out[:].offset,
                        ap=[[0, P], [1, 2]]))

            # trailing max chains fill the collective's latency window
            for j in sorted(pending):
                maxchain(*pending.pop(j))

            # ---- per-row weights (independent of the collective) ----
            yt = persist.tile([P, CP], bf16)
            nc.scalar.activation(yt[:], xt_sb[:], Act.Exp)
            corr = persist.tile([P, CP], bf16)
            nc.vector.tensor_tensor(corr[:], pmax[:], yt[:], op=Op.is_equal)
            valid_b = persist.tile([P, CP], bf16)
            nc.vector.tensor_copy(valid_b[:], valid_sb[:])

            m1 = persist.tile([P, CP], bf16)             # correct: max_alpha
            nc.vector.scalar_tensor_tensor(m1[:], pmax[:], 1.0, corr[:],
                                           op0=Op.add, op1=Op.mult)
            cmv = persist.tile([P, CP], bf16)
            nc.vector.tensor_sub(cmv[:], corr[:], valid_b[:])
            m0 = persist.tile([P, CP], bf16)             # incorrect: 1-max_a
            nc.vector.tensor_mul(m0[:], cmv[:], pmax[:])

            sumalpha = persist.tile([P, CP], f32)
            nc.scalar.activation(sumalpha[:], sumexp[:], Act.Identity,
                                 bias=c100[:])
            rcp = persist.tile([P, CP], f32)             # 1 / (C + sumexp)
            nc.vector.reciprocal(rcp[:], sumalpha[:])
            t_ = persist.tile([P, CP], bf16)             # tanh(C * rcp)
            nc.scalar.activation(t_[:], rcp[:], Act.Tanh, scale=float(C))
            omt = persist.tile([P, CP], bf16)            # 1 - t
            nc.scalar.activation(omt[:], t_[:], Act.Copy, bias=1.0,
                                 scale=-1.0)

            w4 = persist.tile([P, CP, 4], bf16)          # c-major weights
            nc.vector.tensor_mul(w4[:, :, 0], m1[:], omt[:])   # ac
            nc.vector.tensor_mul(w4[:, :, 1], m1[:], t_[:])    # au
            nc.vector.tensor_mul(w4[:, :, 2], m0[:], omt[:])   # ic
            nc.vector.tensor_mul(w4[:, :, 3], m0[:], t_[:])    # iu
            nc.vector.memset(w4[:, COLS:CP, :], 0.0)

            # gmm holds (max_se, -min_se); umax = C/(C+min_se),
            # umin = C/(C+max_se)
            gsa = persist.tile([P, 2], f32)   # (C+max_se, C+min_se)
            nc.vector.tensor_scalar(gsa[:, 0:1], gmm[:, 0:1], float(C), None,
                                    Op.add)
            nc.vector.tensor_scalar(gsa[:, 1:2], gmm[:, 1:2], -1.0, float(C),
                                    Op.mult, Op.add)
            gu = persist.tile([P, 2], f32)    # (umin, umax)
            nc.vector.reciprocal(gu[:], gsa[:])
            nc.vector.tensor_scalar(gu[:], gu[:], float(C), None, Op.mult)
            # bucket b = (unc - umin) * 20 / (umax - umin)
            #          = rcp * (20*C/rng) - umin*20/rng     (unc = C*rcp)
            rng = persist.tile([P, 1], f32)
            nc.vector.tensor_sub(rng[:], gu[:, 1:2], gu[:, 0:1])
            rrng = persist.tile([P, 1], f32)
            nc.vector.reciprocal(rrng[:], rng[:])
            s1v = persist.tile([P, 1], f32)   # 20*C/rng
            nc.vector.tensor_scalar(s1v[:], rrng[:], float((NTH - 1) * C),
                                    None, Op.mult)
            u0s = persist.tile([P, 1], f32)   # umin*20/rng
            nc.vector.tensor_scalar(u0s[:], rrng[:], gu[:, 0:1],
                                    float(NTH - 1), Op.mult, Op.mult)
            bc = persist.tile([P, CP], bf16)
            nc.vector.scalar_tensor_tensor(
                bc[:], rcp[:], s1v[:], u0s[:].broadcast_to((P, CP)),
                op0=Op.mult, op1=Op.subtract)

            # mask[p, g, k, c] = bc[p, g*G+c] <= k: group-major layout keeps
            # the per-k build in 4x mode (innermost [1, G] bf16 writes) while
            # each group's [K, G] block stays contiguous, so the matmul
            # moving AP collapses to one free dim. Matmuls accumulate into a
            # block-diagonal [4G, GK] PSUM: S[c*4+s, k*G+c] += w4*mask.
            S = psump.tile([4 * G, G * K], f32)
            NGMAX = max(CWS) // G
            with tc.tile_pool(name="maskp", bufs=2) as maskp:
                c0 = 0
                for cw in CWS:
                    ng = cw // G
                    mask = maskp.tile([P, NGMAX, K, G], bf16, tag="mask")
                    bcv = bc[:, c0:c0 + cw].rearrange(
                        "p (g c) -> p g c", c=G)
                    for k in range(K):
                        nc.vector.tensor_scalar(
                            mask[:, 0:ng, k, :], bcv, float(k), None,
                            Op.is_le)
                    for gi in range(ng):
                        g0 = c0 + gi * G
                        nc.tensor.matmul(
                            S[:],
                            w4[:, g0:g0 + G, :],
                            mask[:, gi, :, :],
                            start=(g0 == 0), stop=(g0 + G >= CP))
                    c0 += cw

            s_all = persist.tile([4 * G, G * K], f32)
            nc.vector.tensor_copy(s_all[:], S[:])
            # diagonal extraction: n_part[s, c, k] = S[c*4+s, k*G+c].
            # Round-trip S through flat DRAM; the reload's partition stride
            # of 16*GK+4 bakes the per-c-quarter diagonal shift in, so one
            # uniform strided reduce yields the [4, s*K] partials.
            ROW = G * K                                    # 352
            QS = (4 * G // 4) * ROW + 4                    # 5636
            dflat = dram.tile([1, 4 * G * ROW + 16], f32)
            nc.sync.dma_start(
                bass.AP(tensor=dflat.tensor, offset=dflat[:].offset,
                        ap=[[ROW, 4 * G], [1, ROW]]),
                s_all[:])
            s_line = persist.tile([4, (4 * G // 4) * ROW], f32)
            nc.sync.dma_start(
                s_line[:],
                bass.AP(tensor=dflat.tensor, offset=dflat[:].offset,
                        ap=[[QS, 4], [1, (4 * G // 4) * ROW]]))
            s_sb = persist.tile([4, 4 * K], f32)   # per-quarter [s, k] sums
            nc.vector.tensor_reduce(
                s_sb[:],
                bass.AP(tensor=s_line.tensor, offset=s_line[:].offset,
                        ap=[list(s_line[:].ap[0]), [ROW, 4], [G, K],
                            [4 * ROW + 1, 4]]),
                axis=X, op=Op.add)
            cc2_in = dram.tile([4, 4 * K], f32)
            cc2_out = dram.tile([4, 4 * K], f32)
            nc.sync.dma_start(cc2_in[:], s_sb[:])
            nc.gpsimd.collective_compute(
                "AllReduce", Op.add,
                replica_groups=[list(range(N_CORES))],
                ins=[cc2_in[:].opt()], outs=[cc2_out[:].opt()])

            # flatten [4, 4K] -> [1, 16K] on partition 0, sum the quarters
            fcat = persist.tile([1, 16 * K], f32)
            nc.sync.dma_start(
                fcat[:], cc2_out[:].rearrange("a b -> (a b)").unsqueeze(0))
            f = persist.tile([1, 4 * K], f32)
            nc.vector.tensor_reduce(
                f[:],
                bass.AP(tensor=fcat.tensor, offset=fcat[:].offset,
                        ap=[list(fcat[:].ap[0]), [1, 4 * K], [4 * K, 4]]),
                axis=X, op=Op.add)

            ac = f[:, 0:NTH]
            aup = f[:, K:K + NTH]
            t_au = f[:, K + NTH:K + NTH + 1]
            ic = f[:, 2 * K:2 * K + NTH]
            iup = f[:, 3 * K:3 * K + NTH]
            t_iu = f[:, 3 * K + NTH:3 * K + NTH + 1]

            nneg_iu = persist.tile([1, NTH], f32)        # -n_iu
            nc.vector.tensor_scalar(nneg_iu[:], iup, t_iu, None, Op.subtract)
            num = persist.tile([1, NTH], f32)            # n_ac + n_iu
            nc.vector.tensor_sub(num[:], ac, nneg_iu[:])
            nneg_au = persist.tile([1, NTH], f32)        # -n_au
            nc.vector.tensor_scalar(nneg_au[:], aup, t_au, None, Op.subtract)
            den = persist.tile([1, NTH], f32)
            nc.vector.tensor_sub(den[:], num[:], nneg_au[:])
            nc.vector.tensor_add(den[:], den[:], ic)
            nc.vector.tensor_scalar(den[:], den[:], EPS, None, Op.add)
            rden = persist.tile([1, NTH], f32)
            nc.vector.reciprocal(rden[:], den[:])
            evu = persist.tile([1, NTH], f32)
            nc.vector.tensor_mul(evu[:], num[:], rden[:])

            ssum = persist.tile([1, 1], f32)
            nc.vector.reduce_sum(ssum[:], evu[:], axis=X)
            edge = persist.tile([1, 1], f32)
            nc.vector.tensor_add(edge[:], evu[:, 0:1], evu[:, NTH - 1:NTH])
            nc.vector.tensor_scalar(edge[:], edge[:], 0.5, None, Op.mult)
            auc = persist.tile([1, 1], f32)
            nc.vector.tensor_sub(auc[:], ssum[:], edge[:])
            nc.vector.tensor_scalar(auc[:], auc[:], 1.0 / (NTH - 1), None,
                                    Op.mult)
            eps_t = persist.tile([1, 1], f32)
            nc.vector.memset(eps_t[:], EPS)
            nll = persist.tile([1, 1], f32)
            nc.scalar.activation(nll[:], auc[:], Act.Ln, bias=eps_t[:])
            res = persist.tile([1, 1], f32)
            nc.vector.tensor_scalar(res[:], nll[:], -1.0, None, Op.mult)
            nc.sync.dma_start(out_d.ap(), res[:])

    nc.compile()
    return nc


_NC = None


def _get_nc():
    global _NC
    if _NC is None:
        _NC = _build_nc()
    return _NC


_VALID = None


def _valid_mask():
    global _VALID
    if _VALID is None:
        v = np.ones((P, CP), np.float32)
        v[:, MCOLS:] = 0.0
        v[:REM, MCOLS] = 1.0
        _VALID = v
    return _VALID


def _in_maps(output, target):
    output = np.ascontiguousarray(np.asarray(output, dtype=np.float32))
    tgt = np.asarray(target).astype(np.int64)
    xt_full = output[np.arange(output.shape[0]), tgt].astype(np.float32)
    v = _valid_mask()
    maps = []
    for i in range(N_CORES):
        xs = output[i * NPC:(i + 1) * NPC]
        xtc = xt_full[i * NPC:(i + 1) * NPC]
        xtm = np.full((P, CP), -1e30, np.float32)
        xtm[:, :MCOLS] = xtc[:P * MCOLS].reshape(P, MCOLS)
        xtm[:REM, MCOLS] = xtc[P * MCOLS:]
        maps.append({"x": xs, "xt": xtm, "valid": v})
    return maps


def run(output, target, trace=False):
    from concourse.bass_utils import run_bass_kernel_spmd
    nc = _get_nc()
    res = run_bass_kernel_spmd(nc, _in_maps(output, target),
                               core_ids=list(range(N_CORES)), trace=trace)
    val = np.float32(res.results[0]["out"][0, 0])
    return val, res


def kernel(output, target, num_classes):
    assert int(num_classes) == C
    val, _ = run(output, target)
    return np.array(val, dtype=np.float32)
